# revision 8
# baseline (speedup 1.0000x reference)
"""Trainium2 Bass kernel for nn_ContrastiveLoss2 (SimCLR-style NT-Xent loss).

Math (matches the jax reference):
    z  = concat([z_augment, z_orig])                       # [N=8192, D=256]
    zn = z / max(||z||, eps)                               # row L2 normalize
    S  = zn @ zn.T                                         # cosine sim [N, N]
    loss_i = -S[i, i+-B]/tau + log( sum_{j != i} exp(S[i,j]/tau) )
    out = mean_i loss_i                                    # tau = 0.5

Key identity: the softmax denominator is the full row sum of exp(S/tau)
minus the diagonal term exp(S_ii/tau).

Distribution: data-parallel over the 8192 rows -> 1024 rows per core.
Each core receives the full z ROTATED so that its own rows sit at
[0:1024) and the positive partners at [4096:5120).  Pure SPMD, no
collectives; the host sums the 8 per-core partial losses.

Per-core pipeline (engine assignment in parentheses):
  - load z in 8 groups of 1024 rows (SP DMA)
  - per-tile sum-of-squares via fused tensor_tensor_reduce (DVE)
  - 1/norm = exp(-0.5*ln(sumsq)) (ACT, one table set)
  - zn = z * invnorm -> bf16, one fused DVE op per group slice
  - bf16 bounce to DRAM (Pool DMA) + transposed reload (SP xbar DMA)
  - bf16 -> fp8e4 cast of the transposed operand (Pool)
  - S row-blocks via fp8 DoubleRow matmul (PE), K=256 in one pass
  - exp(2*S) + row-sum via activation accum_out (ACT), 2048-col chunks,
    column-major over the sim matrix so the ACT queue never starves
  - loss assembly (DVE/ACT) -> [128, 8] per-row losses -> DRAM

Groups 0/1 are prepped in fine 256-row slices to shorten the pipeline
ramp before the first exp; later groups are prepped 1024 rows at a
time, emitted between exp column-blocks so every engine queue stays
in dependency order.
"""

import sys

import numpy as np

try:
    import concourse  # noqa: F401
except ImportError:  # pragma: no cover
    sys.path.insert(0, "/opt/trn_rl_repo")

N_CORES = 8
N = 8192          # total rows (2B)
D = 256           # feature dim
B = 4096          # batch (positive offset)
ROWS_PER_CORE = N // N_CORES   # 1024
P = 128           # SBUF partitions
NT = N // P       # 64 natural row-tiles
NGRP = 8          # prep groups (1024 rows each)
TPG = NT // NGRP  # 8 tiles per group
NI = ROWS_PER_CORE // P        # 8 own row-tiles
CH = 512          # matmul chunk (one PSUM bank of fp32)
CC = 2048         # ACT exp chunk width = 4 PSUM banks
NCC = N // CC     # 4 column chunks
NCB = NCC + 1     # column blocks (first one split in two)
TAU = 0.5


def _kernel_body(ctx, tc, out_ap, zr_ap):
    import concourse.bass as bass  # noqa: F401
    from concourse import mybir

    nc = tc.nc
    f32 = mybir.dt.float32
    bf16 = mybir.dt.bfloat16
    fp8 = mybir.dt.float8e4
    Fn = mybir.ActivationFunctionType
    Op = mybir.AluOpType

    p_znat = ctx.enter_context(tc.tile_pool(name="znat", bufs=1))
    p_zn = ctx.enter_context(tc.tile_pool(name="zn", bufs=1))
    p_zntb = ctx.enter_context(tc.tile_pool(name="zntb", bufs=1))
    p_stats = ctx.enter_context(tc.tile_pool(name="stats", bufs=1))
    p_sq = ctx.enter_context(tc.tile_pool(name="sq", bufs=2))
    p_ex = ctx.enter_context(tc.tile_pool(name="ex", bufs=2))
    p_ps = ctx.enter_context(tc.tile_pool(name="ps", bufs=2, space="PSUM"))
    p_dram = ctx.enter_context(tc.tile_pool(name="dram", bufs=1, space="DRAM"))

    # per-group tiles: keeps the dependency tracker's byte ranges disjoint
    # so transposes/casts of group g never serialize behind group g+1
    znat = [p_znat.tile([P, TPG * D], f32, tag=f"znat{g}", name=f"znat{g}")
            for g in range(NGRP)]
    zn = [p_zn.tile([P, TPG * D], bf16, tag=f"zn{g}", name=f"zn{g}")
          for g in range(NGRP)]
    zntb = [p_zntb.tile([P, 2, TPG * P], bf16, tag=f"zntb{g}", name=f"zntb{g}")
            for g in range(NGRP)]
    zbounce = [p_dram.tile([TPG * P, D], bf16, tag=f"zb{g}", name=f"zb{g}")
               for g in range(NGRP)]
    ss = p_stats.tile([P, NT], f32, tag="ss")     # per-row sum of squares
    inv = p_stats.tile([P, NT], f32, tag="inv")   # per-row 1/norm
    posr = p_stats.tile([P, NI], f32, tag="posr")  # raw dot(z_i, z_partner)
    sums = p_stats.tile([P, NI * NCB], f32, tag="sums")  # exp row-sum parts

    def prep(g, nsl=1):
        """Prepare group g (1024 rows) in nsl pipeline slices."""
        spt = TPG // nsl          # tiles per slice
        for s in range(nsl):
            t0 = g * TPG + s * spt        # global first tile of slice
            l0 = s * spt                  # tile offset within group tiles
            # ---- load rotated rows [t0*128, (t0+spt)*128) (SP) ----
            nc.sync.dma_start(
                out=znat[g][:, l0 * D:(l0 + spt) * D].rearrange(
                    "p (t c) -> p t c", c=D),
                in_=zr_ap[t0 * P:(t0 + spt) * P, :].rearrange(
                    "(t p) c -> p t c", p=P),
            )

            # ---- per-tile sumsq via fused mul+reduce (DVE) ----
            for t in range(spt):
                a = znat[g][:, (l0 + t) * D:(l0 + t + 1) * D]
                sq = p_sq.tile([P, D], f32, tag="sq", name="sq")
                nc.vector.tensor_tensor_reduce(
                    out=sq[:], in0=a, in1=a,
                    scale=1.0, scalar=0.0,
                    op0=Op.mult, op1=Op.add,
                    accum_out=ss[:, t0 + t:t0 + t + 1],
                )

            # ---- 1/norm = exp(-0.5 * ln(sumsq)); single ACT table set ----
            lns = p_sq.tile([P, spt], f32, tag="lns", name="lns")
            nc.scalar.activation(lns[:], ss[:, t0:t0 + spt], Fn.Ln)
            nc.scalar.activation(
                inv[:, t0:t0 + spt], lns[:], Fn.Exp, scale=-0.5)

            # ---- normalize slice to bf16 in ONE DVE op (broadcast inv) ----
            inv_bc = inv[:, t0:t0 + spt].rearrange(
                "p (t o) -> p t o", o=1).broadcast_to((P, spt, D))
            nc.vector.tensor_mul(
                zn[g][:, l0 * D:(l0 + spt) * D].rearrange(
                    "p (t c) -> p t c", c=D),
                znat[g][:, l0 * D:(l0 + spt) * D].rearrange(
                    "p (t c) -> p t c", c=D),
                inv_bc)

            # ---- bounce bf16 to DRAM (Pool), then 2 transposed xbar
            # ---- loads [rows,128]->[128,rows] (SP), then fp8 cast (Pool) ---
            nc.gpsimd.dma_start(
                out=zbounce[g][l0 * P:(l0 + spt) * P, :].rearrange(
                    "(t p) c -> p t c", p=P),
                in_=zn[g][:, l0 * D:(l0 + spt) * D].rearrange(
                    "p (t c) -> p t c", c=D),
            )
            for k in range(2):
                nc.sync.dma_start_transpose(
                    zntb[g][:, k, l0 * P:(l0 + spt) * P],
                    zbounce[g][l0 * P:(l0 + spt) * P, k * P:(k + 1) * P],
                )

        # ---- raw positive dots: own tiles (g0) x partner tiles (g4) ----
        if g == 4:
            for i in range(NI):
                sq = p_sq.tile([P, D], f32, tag="sq", name="sqp")
                nc.vector.tensor_tensor_reduce(
                    out=sq[:],
                    in0=znat[0][:, i * D:(i + 1) * D],
                    in1=znat[4][:, i * D:(i + 1) * D],
                    scale=1.0, scalar=0.0,
                    op0=Op.mult, op1=Op.add,
                    accum_out=posr[:, i:i + 1],
                )

    # Warm the ACT Ln/Exp table at t=0 so the first real Ln doesn't pay
    # the 1.3us table load on the critical path.
    warm = p_stats.tile([P, 1], f32, tag="warm")
    nc.vector.memset(warm[:], 1.0)
    wrm2 = p_stats.tile([P, 1], f32, tag="warm2")
    nc.scalar.activation(wrm2[:], warm[:], Fn.Ln)
    nc.scalar.activation(wrm2[:], warm[:], Fn.Exp)

    # All preps are emitted up front: the tile scheduler is ready-driven
    # with emission-order priority, so each group's small ACT/DVE ops
    # preempt the long exp stream the moment their inputs land, while
    # the exps (lower priority, but ready) fill the gaps.
    prep(0, nsl=4)
    prep(1, nsl=4)
    for g in range(2, NGRP):
        prep(g)

    # ---- main loop: column-major S row-blocks + fused exp/rowsum ----
    # first two blocks are 1024 wide so the exp stream starts as soon as
    # group 0 alone is transposed; later blocks span 2048 columns
    cblocks = [(0, 1024), (1024, 1024)] + [
        (c, CC) for c in range(2048, N, CC)]
    assert sum(w for _, w in cblocks) == N
    for ci, (cbase, cw) in enumerate(cblocks):
        for t in range(NI):
            ps = p_ps.tile([P, CC], f32, tag="ps", name="psmm")
            for jc in range(cw // CH):
                col = cbase + jc * CH
                g, c0 = divmod(col, TPG * P)
                for k in range(2):
                    nc.tensor.matmul(
                        ps[:, jc * CH:(jc + 1) * CH],
                        lhsT=zntb[0][:, k, t * P:(t + 1) * P],
                        rhs=zntb[g][:, k, c0:c0 + CH],
                        start=(k == 0), stop=(k == 1),
                    )
            ex = p_ex.tile([P, CC], fp8, tag="ex", name="ex")
            nc.scalar.activation(
                ex[:, 0:cw], ps[:, 0:cw], Fn.Exp, scale=2.0,
                accum_out=sums[:, t * NCB + ci:t * NCB + ci + 1],
            )

    # ---- loss assembly: loss = -2*pos + ln(rowsum - exp(2*selfsim)) ----
    totals = p_stats.tile([P, NI], f32, tag="tot")
    nc.vector.tensor_reduce(
        totals[:], sums[:].rearrange("p (i c) -> p i c", c=NCB),
        axis=mybir.AxisListType.X, op=Op.add,
    )
    s1 = p_stats.tile([P, NI], f32, tag="s1")
    nc.vector.tensor_tensor(s1[:], ss[:, 0:NI], inv[:, 0:NI], op=Op.mult)
    s2 = p_stats.tile([P, NI], f32, tag="s2")
    nc.vector.tensor_tensor(s2[:], s1[:], inv[:, 0:NI], op=Op.mult)
    es = p_stats.tile([P, NI], f32, tag="es")
    nc.scalar.activation(es[:], s2[:], Fn.Exp, scale=2.0)
    neg = p_stats.tile([P, NI], f32, tag="neg")
    nc.vector.tensor_sub(neg[:], totals[:], es[:])
    lg = p_stats.tile([P, NI], f32, tag="lg")
    nc.scalar.activation(lg[:], neg[:], Fn.Ln)
    p1 = p_stats.tile([P, NI], f32, tag="p1")
    nc.vector.tensor_tensor(p1[:], posr[:], inv[:, 0:NI], op=Op.mult)
    p2 = p_stats.tile([P, NI], f32, tag="p2")
    nc.vector.tensor_tensor(
        p2[:], p1[:], inv[:, NT // 2:NT // 2 + NI], op=Op.mult)
    loss = p_stats.tile([P, NI], f32, tag="loss")
    nc.vector.scalar_tensor_tensor(
        out=loss[:], in0=p2[:], scalar=-2.0 / (2.0 * TAU), in1=lg[:],
        op0=Op.mult, op1=Op.add,
    )
    nc.sync.dma_start(out=out_ap, in_=loss[:])


def build_nc():
    """Build (once) the Bass module shared by all 8 cores."""
    from contextlib import ExitStack

    from concourse import bacc, mybir
    import concourse.tile as tile

    nc = bacc.Bacc("TRN2", target_bir_lowering=False, debug=False)
    zr = nc.dram_tensor("zr", [N, D], mybir.dt.float32,
                        kind="ExternalInput").ap()
    out = nc.dram_tensor("out", [P, NI], mybir.dt.float32,
                         kind="ExternalOutput").ap()
    with tile.TileContext(nc) as tc:
        with ExitStack() as ctx:
            _kernel_body(ctx, tc, out, zr)
    return nc


_NC = None


def _get_nc(finalized=True):
    global _NC
    if _NC is None:
        _NC = build_nc()
    if finalized and not _NC.is_finalized():
        _NC.finalize()
    return _NC


def make_in_maps(z_orig, z_augment):
    z = np.ascontiguousarray(
        np.concatenate([np.asarray(z_augment, dtype=np.float32),
                        np.asarray(z_orig, dtype=np.float32)], axis=0))
    return [{"zr": np.roll(z, -ROWS_PER_CORE * c, axis=0)}
            for c in range(N_CORES)]


def reduce_outputs(results):
    total = 0.0
    for r in results:
        total += float(np.asarray(r["out"], dtype=np.float64).sum())
    return np.float32(total / N)


def kernel(z_orig, z_augment):
    from concourse.bass_utils import run_bass_kernel_spmd

    nc = _get_nc()
    in_maps = make_in_maps(z_orig, z_augment)
    res = run_bass_kernel_spmd(nc, in_maps, core_ids=list(range(N_CORES)))
    return reduce_outputs(res.results)


# revision 9
# speedup vs baseline: 1.0390x; 1.0390x over previous
"""Trainium2 Bass kernel for nn_ContrastiveLoss2 (SimCLR-style NT-Xent loss).

Math (matches the jax reference):
    z  = concat([z_augment, z_orig])                       # [N=8192, D=256]
    zn = z / max(||z||, eps)                               # row L2 normalize
    S  = zn @ zn.T                                         # cosine sim [N, N]
    loss_i = -S[i, i+-B]/tau + log( sum_{j != i} exp(S[i,j]/tau) )
    out = mean_i loss_i                                    # tau = 0.5

Key identity: the softmax denominator is the full row sum of exp(S/tau)
minus the diagonal term exp(S_ii/tau).

Distribution: data-parallel over the 8192 rows -> 1024 rows per core.
Each core receives the full z ROTATED so that its own rows sit at
[0:1024) and the positive partners at [4096:5120).  Pure SPMD, no
collectives; the host sums the 8 per-core partial losses.

Per-core pipeline (engine assignment in parentheses):
  - load z in 8 groups of 1024 rows (SP DMA)
  - per-tile sum-of-squares via fused tensor_tensor_reduce (DVE)
  - 1/norm = exp(-0.5*ln(sumsq)) (ACT, one table set)
  - zn = z * invnorm -> bf16, one fused DVE op per group slice
  - bf16 bounce to DRAM (Pool DMA) + transposed reload (SP xbar DMA)
  - bf16 -> fp8e4 cast of the transposed operand (Pool)
  - S row-blocks via fp8 DoubleRow matmul (PE), K=256 in one pass
  - exp(2*S) + row-sum via activation accum_out (ACT), 2048-col chunks,
    column-major over the sim matrix so the ACT queue never starves
  - loss assembly (DVE/ACT) -> [128, 8] per-row losses -> DRAM

Groups 0/1 are prepped in fine 256-row slices to shorten the pipeline
ramp before the first exp; later groups are prepped 1024 rows at a
time, emitted between exp column-blocks so every engine queue stays
in dependency order.
"""

import sys

import numpy as np

try:
    import concourse  # noqa: F401
except ImportError:  # pragma: no cover
    sys.path.insert(0, "/opt/trn_rl_repo")

N_CORES = 8
N = 8192          # total rows (2B)
D = 256           # feature dim
B = 4096          # batch (positive offset)
ROWS_PER_CORE = N // N_CORES   # 1024
P = 128           # SBUF partitions
NT = N // P       # 64 natural row-tiles
NGRP = 8          # prep groups (1024 rows each)
TPG = NT // NGRP  # 8 tiles per group
NI = ROWS_PER_CORE // P        # 8 own row-tiles
CH = 512          # matmul chunk (one PSUM bank of fp32)
CC = 2048         # ACT exp chunk width = 4 PSUM banks
NCC = N // CC     # 4 column chunks
NCB = NCC         # column blocks
TAU = 0.5


def _kernel_body(ctx, tc, out_ap, zr_ap):
    import concourse.bass as bass  # noqa: F401
    from concourse import mybir

    nc = tc.nc
    f32 = mybir.dt.float32
    bf16 = mybir.dt.bfloat16
    fp8 = mybir.dt.float8e4
    Fn = mybir.ActivationFunctionType
    Op = mybir.AluOpType

    p_znat = ctx.enter_context(tc.tile_pool(name="znat", bufs=1))
    p_zn = ctx.enter_context(tc.tile_pool(name="zn", bufs=1))
    p_zntb = ctx.enter_context(tc.tile_pool(name="zntb", bufs=1))
    p_stats = ctx.enter_context(tc.tile_pool(name="stats", bufs=1))
    p_sq = ctx.enter_context(tc.tile_pool(name="sq", bufs=2))
    p_ex = ctx.enter_context(tc.tile_pool(name="ex", bufs=2))
    p_ps = ctx.enter_context(tc.tile_pool(name="ps", bufs=2, space="PSUM"))
    p_dram = ctx.enter_context(tc.tile_pool(name="dram", bufs=1, space="DRAM"))

    # per-group tiles: keeps the dependency tracker's byte ranges disjoint
    # so transposes/casts of group g never serialize behind group g+1
    znat = [p_znat.tile([P, TPG * D], f32, tag=f"znat{g}", name=f"znat{g}")
            for g in range(NGRP)]
    zn = [p_zn.tile([P, TPG * D], bf16, tag=f"zn{g}", name=f"zn{g}")
          for g in range(NGRP)]
    zntb = [p_zntb.tile([P, 2, TPG * P], bf16, tag=f"zntb{g}", name=f"zntb{g}")
            for g in range(NGRP)]
    zbounce = [p_dram.tile([TPG * P, D], bf16, tag=f"zb{g}", name=f"zb{g}")
               for g in range(NGRP)]
    ss = p_stats.tile([P, NT], f32, tag="ss")     # per-row sum of squares
    inv = p_stats.tile([P, NT], f32, tag="inv")   # per-row 1/norm
    posr = p_stats.tile([P, NI], f32, tag="posr")  # raw dot(z_i, z_partner)
    sums = p_stats.tile([P, NI * NCB], f32, tag="sums")  # exp row-sum parts

    def prep(g, nsl=1):
        """Prepare group g (1024 rows) in nsl pipeline slices."""
        spt = TPG // nsl          # tiles per slice
        for s in range(nsl):
            t0 = g * TPG + s * spt        # global first tile of slice
            l0 = s * spt                  # tile offset within group tiles
            # ---- load rotated rows [t0*128, (t0+spt)*128) (SP) ----
            nc.sync.dma_start(
                out=znat[g][:, l0 * D:(l0 + spt) * D].rearrange(
                    "p (t c) -> p t c", c=D),
                in_=zr_ap[t0 * P:(t0 + spt) * P, :].rearrange(
                    "(t p) c -> p t c", p=P),
            )

            # ---- per-tile sumsq via fused mul+reduce (DVE) ----
            for t in range(spt):
                a = znat[g][:, (l0 + t) * D:(l0 + t + 1) * D]
                sq = p_sq.tile([P, D], f32, tag="sq", name="sq")
                nc.vector.tensor_tensor_reduce(
                    out=sq[:], in0=a, in1=a,
                    scale=1.0, scalar=0.0,
                    op0=Op.mult, op1=Op.add,
                    accum_out=ss[:, t0 + t:t0 + t + 1],
                )

            # ---- 1/norm = exp(-0.5 * ln(sumsq)); single ACT table set ----
            lns = p_sq.tile([P, spt], f32, tag="lns", name="lns")
            nc.scalar.activation(lns[:], ss[:, t0:t0 + spt], Fn.Ln)
            nc.scalar.activation(
                inv[:, t0:t0 + spt], lns[:], Fn.Exp, scale=-0.5)

            # ---- normalize slice to bf16 in ONE DVE op (broadcast inv) ----
            inv_bc = inv[:, t0:t0 + spt].rearrange(
                "p (t o) -> p t o", o=1).broadcast_to((P, spt, D))
            nc.vector.tensor_mul(
                zn[g][:, l0 * D:(l0 + spt) * D].rearrange(
                    "p (t c) -> p t c", c=D),
                znat[g][:, l0 * D:(l0 + spt) * D].rearrange(
                    "p (t c) -> p t c", c=D),
                inv_bc)

            # ---- bounce bf16 to DRAM (Pool), then 2 transposed xbar
            # ---- loads [rows,128]->[128,rows] (SP), then fp8 cast (Pool) ---
            nc.gpsimd.dma_start(
                out=zbounce[g][l0 * P:(l0 + spt) * P, :].rearrange(
                    "(t p) c -> p t c", p=P),
                in_=zn[g][:, l0 * D:(l0 + spt) * D].rearrange(
                    "p (t c) -> p t c", c=D),
            )
            for k in range(2):
                nc.sync.dma_start_transpose(
                    zntb[g][:, k, l0 * P:(l0 + spt) * P],
                    zbounce[g][l0 * P:(l0 + spt) * P, k * P:(k + 1) * P],
                )

        # ---- raw positive dots: own tiles (g0) x partner tiles (g4) ----
        if g == 4:
            for i in range(NI):
                sq = p_sq.tile([P, D], f32, tag="sq", name="sqp")
                nc.vector.tensor_tensor_reduce(
                    out=sq[:],
                    in0=znat[0][:, i * D:(i + 1) * D],
                    in1=znat[4][:, i * D:(i + 1) * D],
                    scale=1.0, scalar=0.0,
                    op0=Op.mult, op1=Op.add,
                    accum_out=posr[:, i:i + 1],
                )

    # Warm the ACT Ln/Exp table at t=0 so the first real Ln doesn't pay
    # the 1.3us table load on the critical path.
    warm = p_stats.tile([P, 1], f32, tag="warm")
    nc.vector.memset(warm[:], 1.0)
    wrm2 = p_stats.tile([P, 1], f32, tag="warm2")
    nc.scalar.activation(wrm2[:], warm[:], Fn.Ln)
    nc.scalar.activation(wrm2[:], warm[:], Fn.Exp)

    # All preps are emitted up front: the tile scheduler is ready-driven
    # with emission-order priority, so each group's small ACT/DVE ops
    # preempt the long exp stream the moment their inputs land, while
    # the exps (lower priority, but ready) fill the gaps.
    prep(0, nsl=4)
    prep(1, nsl=4)
    for g in range(2, NGRP):
        prep(g)

    # ---- main loop: column-major S row-blocks + fused exp/rowsum ----
    cblocks = [(c, CC) for c in range(0, N, CC)]
    for ci, (cbase, cw) in enumerate(cblocks):
        for t in range(NI):
            ps = p_ps.tile([P, CC], f32, tag="ps", name="psmm")
            for jc in range(cw // CH):
                col = cbase + jc * CH
                g, c0 = divmod(col, TPG * P)
                for k in range(2):
                    nc.tensor.matmul(
                        ps[:, jc * CH:(jc + 1) * CH],
                        lhsT=zntb[0][:, k, t * P:(t + 1) * P],
                        rhs=zntb[g][:, k, c0:c0 + CH],
                        start=(k == 0), stop=(k == 1),
                    )
            ex = p_ex.tile([P, CC], fp8, tag="ex", name="ex")
            nc.scalar.activation(
                ex[:, 0:cw], ps[:, 0:cw], Fn.Exp, scale=2.0,
                accum_out=sums[:, t * NCB + ci:t * NCB + ci + 1],
            )

    # ---- loss assembly: loss = -2*pos + ln(rowsum - exp(2*selfsim)) ----
    totals = p_stats.tile([P, NI], f32, tag="tot")
    nc.vector.tensor_reduce(
        totals[:], sums[:].rearrange("p (i c) -> p i c", c=NCB),
        axis=mybir.AxisListType.X, op=Op.add,
    )
    s1 = p_stats.tile([P, NI], f32, tag="s1")
    nc.vector.tensor_tensor(s1[:], ss[:, 0:NI], inv[:, 0:NI], op=Op.mult)
    s2 = p_stats.tile([P, NI], f32, tag="s2")
    nc.vector.tensor_tensor(s2[:], s1[:], inv[:, 0:NI], op=Op.mult)
    es = p_stats.tile([P, NI], f32, tag="es")
    nc.scalar.activation(es[:], s2[:], Fn.Exp, scale=2.0)
    neg = p_stats.tile([P, NI], f32, tag="neg")
    nc.vector.tensor_sub(neg[:], totals[:], es[:])
    lg = p_stats.tile([P, NI], f32, tag="lg")
    nc.scalar.activation(lg[:], neg[:], Fn.Ln)
    p1 = p_stats.tile([P, NI], f32, tag="p1")
    nc.vector.tensor_tensor(p1[:], posr[:], inv[:, 0:NI], op=Op.mult)
    p2 = p_stats.tile([P, NI], f32, tag="p2")
    nc.vector.tensor_tensor(
        p2[:], p1[:], inv[:, NT // 2:NT // 2 + NI], op=Op.mult)
    loss = p_stats.tile([P, NI], f32, tag="loss")
    nc.vector.scalar_tensor_tensor(
        out=loss[:], in0=p2[:], scalar=-2.0 / (2.0 * TAU), in1=lg[:],
        op0=Op.mult, op1=Op.add,
    )
    nc.sync.dma_start(out=out_ap, in_=loss[:])


def build_nc():
    """Build (once) the Bass module shared by all 8 cores."""
    from contextlib import ExitStack

    from concourse import bacc, mybir
    import concourse.tile as tile

    nc = bacc.Bacc("TRN2", target_bir_lowering=False, debug=False)
    zr = nc.dram_tensor("zr", [N, D], mybir.dt.float32,
                        kind="ExternalInput").ap()
    out = nc.dram_tensor("out", [P, NI], mybir.dt.float32,
                         kind="ExternalOutput").ap()
    with tile.TileContext(nc) as tc:
        with ExitStack() as ctx:
            _kernel_body(ctx, tc, out, zr)
    return nc


_NC = None


def _get_nc(finalized=True):
    global _NC
    if _NC is None:
        _NC = build_nc()
    if finalized and not _NC.is_finalized():
        _NC.finalize()
    return _NC


def make_in_maps(z_orig, z_augment):
    z = np.ascontiguousarray(
        np.concatenate([np.asarray(z_augment, dtype=np.float32),
                        np.asarray(z_orig, dtype=np.float32)], axis=0))
    return [{"zr": np.roll(z, -ROWS_PER_CORE * c, axis=0)}
            for c in range(N_CORES)]


def reduce_outputs(results):
    total = 0.0
    for r in results:
        total += float(np.asarray(r["out"], dtype=np.float64).sum())
    return np.float32(total / N)


def kernel(z_orig, z_augment):
    from concourse.bass_utils import run_bass_kernel_spmd

    nc = _get_nc()
    in_maps = make_in_maps(z_orig, z_augment)
    res = run_bass_kernel_spmd(nc, in_maps, core_ids=list(range(N_CORES)))
    return reduce_outputs(res.results)


# revision 10
# speedup vs baseline: 1.5193x; 1.4623x over previous
"""Trainium2 Bass kernel for nn_ContrastiveLoss2 (SimCLR-style NT-Xent loss).

Math (matches the jax reference):
    z  = concat([z_augment, z_orig])                       # [N=8192, D=256]
    zn = z / max(||z||, eps)                               # row L2 normalize
    S  = zn @ zn.T                                         # cosine sim [N, N]
    loss_i = -S[i, i+-B]/tau + log( sum_{j != i} exp(S[i,j]/tau) )
    out = mean_i loss_i                                    # tau = 0.5

Key identity: the softmax denominator is the full row sum of exp(S/tau)
minus the diagonal term exp(S_ii/tau).

Distribution: data-parallel over the 8192 rows -> 1024 rows per core.
Each core receives the full z ROTATED so that its own rows sit at
[0:1024) and the positive partners at [4096:5120).  Pure SPMD, no
collectives; the host sums the 8 per-core partial losses.

Per-core pipeline (engine assignment in parentheses):
  - load z in 8 groups of 1024 rows (SP DMA)
  - per-tile sum-of-squares via fused tensor_tensor_reduce (DVE)
  - 1/norm = exp(-0.5*ln(sumsq)) (ACT, one table set)
  - zn = z * invnorm -> bf16, one fused DVE op per group slice
  - bf16 bounce to DRAM (Pool DMA) + transposed reload (SP xbar DMA)
  - bf16 -> fp8e4 cast of the transposed operand (Pool)
  - S row-blocks via fp8 DoubleRow matmul (PE), K=256 in one pass
  - exp(2*S) + row-sum via activation accum_out (ACT), 2048-col chunks,
    column-major over the sim matrix so the ACT queue never starves
  - loss assembly (DVE/ACT) -> [128, 8] per-row losses -> DRAM

Groups 0/1 are prepped in fine 256-row slices to shorten the pipeline
ramp before the first exp; later groups are prepped 1024 rows at a
time, emitted between exp column-blocks so every engine queue stays
in dependency order.
"""

import sys

import numpy as np

try:
    import concourse  # noqa: F401
except ImportError:  # pragma: no cover
    sys.path.insert(0, "/opt/trn_rl_repo")

N_CORES = 8
N = 8192          # total rows (2B)
D = 256           # feature dim
B = 4096          # batch (positive offset)
ROWS_PER_CORE = N // N_CORES   # 1024
P = 128           # SBUF partitions
NT = N // P       # 64 natural row-tiles
NGRP = 8          # prep groups (1024 rows each)
TPG = NT // NGRP  # 8 tiles per group
NI = ROWS_PER_CORE // P        # 8 own row-tiles
CH = 512          # matmul chunk (one PSUM bank of fp32)
CC = 2048         # ACT exp chunk width = 4 PSUM banks
NCC = N // CC     # 4 column chunks
NCB = NCC         # column blocks
TAU = 0.5


def _kernel_body(ctx, tc, out_ap, zr_ap):
    import concourse.bass as bass  # noqa: F401
    from concourse import mybir

    nc = tc.nc
    f32 = mybir.dt.float32
    bf16 = mybir.dt.bfloat16
    fp8 = mybir.dt.float8e4
    Fn = mybir.ActivationFunctionType
    Op = mybir.AluOpType

    p_znat = ctx.enter_context(tc.tile_pool(name="znat", bufs=1))
    p_zn = ctx.enter_context(tc.tile_pool(name="zn", bufs=1))
    p_zntb = ctx.enter_context(tc.tile_pool(name="zntb", bufs=1))
    p_stats = ctx.enter_context(tc.tile_pool(name="stats", bufs=1))
    p_sq = ctx.enter_context(tc.tile_pool(name="sq", bufs=2))
    p_ex = ctx.enter_context(tc.tile_pool(name="ex", bufs=2))
    p_ps = ctx.enter_context(tc.tile_pool(name="ps", bufs=2, space="PSUM"))
    p_dram = ctx.enter_context(tc.tile_pool(name="dram", bufs=1, space="DRAM"))

    # per-group tiles: keeps the dependency tracker's byte ranges disjoint
    # so transposes/casts of group g never serialize behind group g+1
    znat = [p_znat.tile([P, TPG * D], f32, tag=f"znat{g}", name=f"znat{g}")
            for g in range(NGRP)]
    zn = [p_zn.tile([P, TPG * D], bf16, tag=f"zn{g}", name=f"zn{g}")
          for g in range(NGRP)]
    zntb = [p_zntb.tile([P, 2, TPG * P], bf16, tag=f"zntb{g}", name=f"zntb{g}")
            for g in range(NGRP)]
    zbounce = [p_dram.tile([TPG * P, D], bf16, tag=f"zb{g}", name=f"zb{g}")
               for g in range(NGRP)]
    ss = p_stats.tile([P, NT], f32, tag="ss")     # per-row sum of squares
    inv = p_stats.tile([P, NT], f32, tag="inv")   # per-row 1/norm
    posr = p_stats.tile([P, NI], f32, tag="posr")  # raw dot(z_i, z_partner)
    sums = p_stats.tile([P, NI * NCB], f32, tag="sums")  # exp row-sum parts

    def prep(g, nsl=1):
        """Prepare group g (1024 rows) in nsl pipeline slices."""
        spt = TPG // nsl          # tiles per slice
        for s in range(nsl):
            t0 = g * TPG + s * spt        # global first tile of slice
            l0 = s * spt                  # tile offset within group tiles
            # ---- load rotated rows [t0*128, (t0+spt)*128) (SP) ----
            nc.sync.dma_start(
                out=znat[g][:, l0 * D:(l0 + spt) * D].rearrange(
                    "p (t c) -> p t c", c=D),
                in_=zr_ap[t0 * P:(t0 + spt) * P, :].rearrange(
                    "(t p) c -> p t c", p=P),
            )

            # ---- per-tile sumsq via fused mul+reduce (DVE) ----
            for t in range(spt):
                a = znat[g][:, (l0 + t) * D:(l0 + t + 1) * D]
                sq = p_sq.tile([P, D], f32, tag="sq", name="sq")
                nc.vector.tensor_tensor_reduce(
                    out=sq[:], in0=a, in1=a,
                    scale=1.0, scalar=0.0,
                    op0=Op.mult, op1=Op.add,
                    accum_out=ss[:, t0 + t:t0 + t + 1],
                )

            # ---- 1/norm = exp(-0.5 * ln(sumsq)); single ACT table set ----
            lns = p_sq.tile([P, spt], f32, tag="lns", name="lns")
            nc.scalar.activation(lns[:], ss[:, t0:t0 + spt], Fn.Ln)
            nc.scalar.activation(
                inv[:, t0:t0 + spt], lns[:], Fn.Exp, scale=-0.5)

            # ---- normalize slice to bf16 in ONE DVE op (broadcast inv) ----
            inv_bc = inv[:, t0:t0 + spt].rearrange(
                "p (t o) -> p t o", o=1).broadcast_to((P, spt, D))
            nc.vector.tensor_mul(
                zn[g][:, l0 * D:(l0 + spt) * D].rearrange(
                    "p (t c) -> p t c", c=D),
                znat[g][:, l0 * D:(l0 + spt) * D].rearrange(
                    "p (t c) -> p t c", c=D),
                inv_bc)

            # ---- bounce bf16 to DRAM (Pool), then 2 transposed xbar
            # ---- loads [rows,128]->[128,rows] (SP), then fp8 cast (Pool) ---
            nc.gpsimd.dma_start(
                out=zbounce[g][l0 * P:(l0 + spt) * P, :].rearrange(
                    "(t p) c -> p t c", p=P),
                in_=zn[g][:, l0 * D:(l0 + spt) * D].rearrange(
                    "p (t c) -> p t c", c=D),
            )
            for k in range(2):
                nc.sync.dma_start_transpose(
                    zntb[g][:, k, l0 * P:(l0 + spt) * P],
                    zbounce[g][l0 * P:(l0 + spt) * P, k * P:(k + 1) * P],
                )

        # ---- raw positive dots: own tiles (g0) x partner tiles (g4) ----
        if g == 4:
            for i in range(NI):
                sq = p_sq.tile([P, D], f32, tag="sq", name="sqp")
                nc.vector.tensor_tensor_reduce(
                    out=sq[:],
                    in0=znat[0][:, i * D:(i + 1) * D],
                    in1=znat[4][:, i * D:(i + 1) * D],
                    scale=1.0, scalar=0.0,
                    op0=Op.mult, op1=Op.add,
                    accum_out=posr[:, i:i + 1],
                )

    # Warm the ACT Ln/Exp table at t=0 so the first real Ln doesn't pay
    # the 1.3us table load on the critical path.
    warm = p_stats.tile([P, 1], f32, tag="warm")
    nc.vector.memset(warm[:], 1.0)
    wrm2 = p_stats.tile([P, 1], f32, tag="warm2")
    nc.scalar.activation(wrm2[:], warm[:], Fn.Ln)
    nc.scalar.activation(wrm2[:], warm[:], Fn.Exp)

    # All preps are emitted up front: the tile scheduler is ready-driven
    # with emission-order priority, so each group's small ACT/DVE ops
    # preempt the long exp stream the moment their inputs land, while
    # the exps (lower priority, but ready) fill the gaps.
    prep(0, nsl=4)
    prep(1, nsl=2)
    for g in range(2, NGRP):
        prep(g)

    # ---- main loop: column-major S row-blocks + fused exp/rowsum ----
    cblocks = [(c, CC) for c in range(0, N, CC)]
    for ci, (cbase, cw) in enumerate(cblocks):
        for t in range(NI):
            ps = p_ps.tile([P, CC], f32, tag="ps", name="psmm")
            for jc in range(cw // CH):
                col = cbase + jc * CH
                g, c0 = divmod(col, TPG * P)
                for k in range(2):
                    nc.tensor.matmul(
                        ps[:, jc * CH:(jc + 1) * CH],
                        lhsT=zntb[0][:, k, t * P:(t + 1) * P],
                        rhs=zntb[g][:, k, c0:c0 + CH],
                        start=(k == 0), stop=(k == 1),
                    )
            ex = p_ex.tile([P, CC], fp8, tag="ex", name="ex")
            nc.scalar.activation(
                ex[:, 0:cw], ps[:, 0:cw], Fn.Exp, scale=2.0,
                accum_out=sums[:, t * NCB + ci:t * NCB + ci + 1],
            )

    # ---- loss assembly: loss = -2*pos + ln(rowsum - exp(2*selfsim)) ----
    totals = p_stats.tile([P, NI], f32, tag="tot")
    nc.vector.tensor_reduce(
        totals[:], sums[:].rearrange("p (i c) -> p i c", c=NCB),
        axis=mybir.AxisListType.X, op=Op.add,
    )
    s1 = p_stats.tile([P, NI], f32, tag="s1")
    nc.vector.tensor_tensor(s1[:], ss[:, 0:NI], inv[:, 0:NI], op=Op.mult)
    s2 = p_stats.tile([P, NI], f32, tag="s2")
    nc.vector.tensor_tensor(s2[:], s1[:], inv[:, 0:NI], op=Op.mult)
    es = p_stats.tile([P, NI], f32, tag="es")
    nc.scalar.activation(es[:], s2[:], Fn.Exp, scale=2.0)
    neg = p_stats.tile([P, NI], f32, tag="neg")
    nc.vector.tensor_sub(neg[:], totals[:], es[:])
    lg = p_stats.tile([P, NI], f32, tag="lg")
    nc.scalar.activation(lg[:], neg[:], Fn.Ln)
    p1 = p_stats.tile([P, NI], f32, tag="p1")
    nc.vector.tensor_tensor(p1[:], posr[:], inv[:, 0:NI], op=Op.mult)
    p2 = p_stats.tile([P, NI], f32, tag="p2")
    nc.vector.tensor_tensor(
        p2[:], p1[:], inv[:, NT // 2:NT // 2 + NI], op=Op.mult)
    loss = p_stats.tile([P, NI], f32, tag="loss")
    nc.vector.scalar_tensor_tensor(
        out=loss[:], in0=p2[:], scalar=-2.0 / (2.0 * TAU), in1=lg[:],
        op0=Op.mult, op1=Op.add,
    )
    nc.sync.dma_start(out=out_ap, in_=loss[:])


def build_nc():
    """Build (once) the Bass module shared by all 8 cores."""
    from contextlib import ExitStack

    from concourse import bacc, mybir
    import concourse.tile as tile

    nc = bacc.Bacc("TRN2", target_bir_lowering=False, debug=False)
    zr = nc.dram_tensor("zr", [N, D], mybir.dt.float32,
                        kind="ExternalInput").ap()
    out = nc.dram_tensor("out", [P, NI], mybir.dt.float32,
                         kind="ExternalOutput").ap()
    with tile.TileContext(nc) as tc:
        with ExitStack() as ctx:
            _kernel_body(ctx, tc, out, zr)
    return nc


_NC = None


def _get_nc(finalized=True):
    global _NC
    if _NC is None:
        _NC = build_nc()
    if finalized and not _NC.is_finalized():
        _NC.finalize()
    return _NC


def make_in_maps(z_orig, z_augment):
    z = np.ascontiguousarray(
        np.concatenate([np.asarray(z_augment, dtype=np.float32),
                        np.asarray(z_orig, dtype=np.float32)], axis=0))
    return [{"zr": np.roll(z, -ROWS_PER_CORE * c, axis=0)}
            for c in range(N_CORES)]


def reduce_outputs(results):
    total = 0.0
    for r in results:
        total += float(np.asarray(r["out"], dtype=np.float64).sum())
    return np.float32(total / N)


def kernel(z_orig, z_augment):
    from concourse.bass_utils import run_bass_kernel_spmd

    nc = _get_nc()
    in_maps = make_in_maps(z_orig, z_augment)
    res = run_bass_kernel_spmd(nc, in_maps, core_ids=list(range(N_CORES)))
    return reduce_outputs(res.results)


# revision 11
# speedup vs baseline: 1.5607x; 1.0273x over previous
"""Trainium2 Bass kernel for nn_ContrastiveLoss2 — symmetric-half variant.

Same math as kernel.py, but exploits the symmetry of E = exp(2*S):
row sums of E equal column sums.  Each core computes E only for the
column bands j = 0..4 relative to its own row band (5/8 of the full
row block instead of 8/8).  Bands j=1..3 are computed exactly once
fleet-wide (the transposed copy is recovered from column sums); band
j=4 is computed twice (once by each end) so every core's row sums
stay self-contained for it; band j=0 is the diagonal block.

Per core outputs:
  - rowpart[r]  = sum over local columns [0:5120) of E[r, :]
  - colpart[n]  = sum over the core's 1024 rows of E[:, n] for local
    columns n in [1024:4096)   (bands j=1..3, shipped to the host)
  - pos[r], selfexp[r]
The host reassembles full row sums (rowpart of band-owner + colparts
from the three cores that computed the transposed blocks), then
finishes loss = mean(-2*pos + ln(rowsum - selfexp)) in float64 --
a [8192]-element gather/sum, all heavy compute stays on device.

Column sums are taken on the PE: the exp values are stored as fp8e4
and contracted with an all-ones fp8 matrix in DoubleRow mode (two
row-tiles per matmul), accumulating over the 8 row-tiles in PSUM.
"""

import sys

import numpy as np

try:
    import concourse  # noqa: F401
except ImportError:  # pragma: no cover
    sys.path.insert(0, "/opt/trn_rl_repo")

N_CORES = 8
N = 8192          # total rows (2B)
D = 256           # feature dim
B = 4096          # batch (positive offset)
ROWS_PER_CORE = N // N_CORES   # 1024
P = 128           # SBUF partitions
NGRP = 5          # column bands computed per core
NT = NGRP * 8     # natural row-tiles loaded (40)
TPG = 8           # tiles per group
NI = ROWS_PER_CORE // P        # 8 own row-tiles
CH = 512          # matmul chunk (one PSUM bank of fp32)
CC = 2048         # ACT exp chunk width = 4 PSUM banks
NCOL = NGRP * TPG * P          # 5120 columns per core
NCB = 3           # column blocks per row-tile
CSPAN = (1024, 4096)           # columns whose colparts ship to host
TAU = 0.5


def _kernel_body(ctx, tc, out_ap, colp_ap, zr_ap):
    import concourse.bass as bass  # noqa: F401
    from concourse import mybir

    nc = tc.nc
    f32 = mybir.dt.float32
    bf16 = mybir.dt.bfloat16
    fp8 = mybir.dt.float8e4
    Fn = mybir.ActivationFunctionType
    Op = mybir.AluOpType

    p_znat = ctx.enter_context(tc.tile_pool(name="znat", bufs=1))
    p_zn = ctx.enter_context(tc.tile_pool(name="zn", bufs=1))
    p_zntb = ctx.enter_context(tc.tile_pool(name="zntb", bufs=1))
    p_stats = ctx.enter_context(tc.tile_pool(name="stats", bufs=1))
    p_sq = ctx.enter_context(tc.tile_pool(name="sq", bufs=2))
    p_ex = ctx.enter_context(tc.tile_pool(name="ex", bufs=1))
    p_ps = ctx.enter_context(tc.tile_pool(name="ps", bufs=2, space="PSUM"))

    znat = [p_znat.tile([P, TPG * D], f32, tag=f"znat{g}", name=f"znat{g}")
            for g in range(NGRP)]
    zn = [p_zn.tile([P, TPG * D], bf16, tag=f"zn{g}", name=f"zn{g}")
          for g in range(NGRP)]
    zntb = [p_zntb.tile([P, 2, TPG * P], bf16, tag=f"zntb{g}", name=f"zntb{g}")
            for g in range(NGRP)]
    # exp values for column-sum recovery: one tile per row-tile PAIR so the
    # ones-matmul can contract 256 rows per DoubleRow pass
    expair = [p_ex.tile([P, 2, NCOL], fp8, tag=f"exp{tp}", name=f"exp{tp}")
              for tp in range(NI // 2)]
    ones = p_ex.tile([P, 2, P], fp8, tag="ones")
    colb = p_ex.tile([P, CSPAN[1] - CSPAN[0]], bf16, tag="colb")
    ss = p_stats.tile([P, NT], f32, tag="ss")     # per-row sum of squares
    inv = p_stats.tile([P, NT], f32, tag="inv")   # per-row 1/norm
    posr = p_stats.tile([P, NI], f32, tag="posr")  # raw dot(z_i, z_partner)
    # packed output: rowsum parts [0:24), pos [24:32), selfexp [32:40)
    sums = p_stats.tile([P, NI * NCB + 2 * NI], f32, tag="sums")

    def prep(g, nsl=1):
        """Prepare group g (1024 rows/columns) in nsl pipeline slices.

        Loads + sumsq for all slices come first, then ONE 1/norm pair on
        ACT for the whole group, then per-slice normalize + transpose --
        a single ACT<->DVE round-trip instead of one per slice.
        """
        spt = TPG // nsl          # tiles per slice
        g0 = g * TPG
        for s in range(nsl):
            t0 = g0 + s * spt             # global first tile of slice
            l0 = s * spt                  # tile offset within group tiles
            # ---- load rotated rows [t0*128, (t0+spt)*128) ----
            ldq = nc.sync if g % 2 == 0 else nc.gpsimd
            ldq.dma_start(
                out=znat[g][:, l0 * D:(l0 + spt) * D].rearrange(
                    "p (t c) -> p t c", c=D),
                in_=zr_ap[t0 * P:(t0 + spt) * P, :].rearrange(
                    "(t p) c -> p t c", p=P),
            )

            # ---- slice sumsq: square then per-tile reduce (DVE) ----
            zg = znat[g][:, l0 * D:(l0 + spt) * D]
            sq = p_sq.tile([P, TPG * D], f32, tag="sq", name="sq")
            nc.vector.tensor_mul(sq[:, 0:spt * D], zg, zg)
            nc.vector.tensor_reduce(
                ss[:, t0:t0 + spt],
                sq[:, 0:spt * D].rearrange("p (t c) -> p t c", c=D),
                axis=mybir.AxisListType.X, op=Op.add)

        # ---- 1/norm = exp(-0.5 * ln(sumsq)); single ACT table set ----
        lns = p_sq.tile([P, TPG], f32, tag="lns", name="lns")
        nc.scalar.activation(lns[:], ss[:, g0:g0 + TPG], Fn.Ln)
        nc.scalar.activation(
            inv[:, g0:g0 + TPG], lns[:], Fn.Exp, scale=-0.5)

        for s in range(nsl):
            t0 = g0 + s * spt
            l0 = s * spt
            # ---- normalize slice to bf16 in ONE DVE op (broadcast inv) ----
            inv_bc = inv[:, t0:t0 + spt].rearrange(
                "p (t o) -> p t o", o=1).broadcast_to((P, spt, D))
            nc.vector.tensor_mul(
                zn[g][:, l0 * D:(l0 + spt) * D].rearrange(
                    "p (t c) -> p t c", c=D),
                znat[g][:, l0 * D:(l0 + spt) * D].rearrange(
                    "p (t c) -> p t c", c=D),
                inv_bc)

            # ---- transpose each [128,128] block via xbar DMA, SBUF to
            # ---- SBUF, no DRAM bounce (SP) ----
            for t in range(spt):
                for k in range(2):
                    nc.sync.dma_start_transpose(
                        zntb[g][:, k, (l0 + t) * P:(l0 + t + 1) * P],
                        zn[g][:, (l0 + t) * D + k * P:
                              (l0 + t) * D + (k + 1) * P],
                    )

        # ---- raw positive dots: own tiles (g0) x partner tiles (g4) ----
        if g == 4:
            sqp = p_sq.tile([P, TPG * D], f32, tag="sq", name="sqp")
            nc.vector.tensor_mul(sqp[:], znat[0][:], znat[4][:])
            nc.vector.tensor_reduce(
                posr[:], sqp[:].rearrange("p (t c) -> p t c", c=D),
                axis=mybir.AxisListType.X, op=Op.add)

    # Warm the ACT Ln/Exp table at t=0 so the first real Ln doesn't pay
    # the 1.3us table load on the critical path; fill the all-ones fp8
    # matrix for the colsum contraction while DVE is idle anyway.
    warm = p_stats.tile([P, 1], f32, tag="warm")
    nc.vector.memset(warm[:], 1.0)
    wrm2 = p_stats.tile([P, 1], f32, tag="warm2")
    nc.scalar.activation(wrm2[:], warm[:], Fn.Ln)
    nc.scalar.activation(wrm2[:], warm[:], Fn.Exp)
    nc.vector.memset(ones[:], 1.0)

    prep(0, nsl=4)
    prep(1, nsl=2)
    for g in range(2, NGRP):
        prep(g)

    # ---- main loop: column-major S row-blocks + fused exp/rowsum.
    # After each block's exps, the column sums for its share of bands
    # j=1..3 are recovered with fp8 DoubleRow ones-matmuls (PE), so the
    # whole colpart pipeline drains during the last block's exps.
    cblocks = [(0, 1024), (1024, 2048), (3072, 2048)]

    def colsum_chains(cbase, cw):
        # recover column sums of E for this block's share of bands j=1..3,
        # then ship them while later blocks are still running
        lo = max(cbase, CSPAN[0])
        hi = min(cbase + cw, CSPAN[1])
        for c0 in range(lo, hi, CH):
            m = (c0 - CSPAN[0]) // CH
            psc = p_ps.tile([P, CC], f32, tag="ps", name="pscol")
            for tp in range(NI // 2):
                nc.tensor.matmul(
                    psc[:, 0:CH],
                    lhsT=ones[:],
                    rhs=expair[tp][:, :, c0:c0 + CH],
                    start=(tp == 0), stop=(tp == NI // 2 - 1),
                    perf_mode=mybir.MatmulPerfMode.DoubleRow,
                )
            nc.vector.tensor_copy(
                out=colb[:, m * CH:(m + 1) * CH], in_=psc[:, 0:CH])
        if hi > lo:
            nc.sync.dma_start(
                out=colp_ap[:, lo - CSPAN[0]:hi - CSPAN[0]],
                in_=colb[0:1, lo - CSPAN[0]:hi - CSPAN[0]])

    for ci, (cbase, cw) in enumerate(cblocks):
        for t in range(NI):
            ps = p_ps.tile([P, CC], f32, tag="ps", name="psmm")
            for jc in range(cw // CH):
                col = cbase + jc * CH
                g, c0 = divmod(col, TPG * P)
                for k in range(2):
                    nc.tensor.matmul(
                        ps[:, jc * CH:(jc + 1) * CH],
                        lhsT=zntb[0][:, k, t * P:(t + 1) * P],
                        rhs=zntb[g][:, k, c0:c0 + CH],
                        start=(k == 0), stop=(k == 1),
                    )
            nc.scalar.activation(
                expair[t // 2][:, t % 2, cbase:cbase + cw],
                ps[:, 0:cw], Fn.Exp, scale=2.0,
                accum_out=sums[:, t * NCB + ci:t * NCB + ci + 1],
            )
        # the PREVIOUS block's chains: emitted after this block's exps so
        # the PSUM slot rotation never stalls the ACT stream
        if ci >= 1:
            colsum_chains(*cblocks[ci - 1])
    colsum_chains(*cblocks[-1])

    # ---- pos & selfexp terms into the packed output ----
    s1 = p_stats.tile([P, NI], f32, tag="s1")
    nc.vector.tensor_tensor(s1[:], ss[:, 0:NI], inv[:, 0:NI], op=Op.mult)
    s2 = p_stats.tile([P, NI], f32, tag="s2")
    nc.vector.tensor_tensor(s2[:], s1[:], inv[:, 0:NI], op=Op.mult)
    nc.scalar.activation(
        sums[:, NI * NCB + NI:NI * NCB + 2 * NI], s2[:], Fn.Exp, scale=2.0)
    p1 = p_stats.tile([P, NI], f32, tag="p1")
    nc.vector.tensor_tensor(p1[:], posr[:], inv[:, 0:NI], op=Op.mult)
    nc.vector.tensor_tensor(
        sums[:, NI * NCB:NI * NCB + NI], p1[:],
        inv[:, 4 * TPG:4 * TPG + NI], op=Op.mult)

    nc.sync.dma_start(out=out_ap, in_=sums[:])


def build_nc():
    """Build (once) the Bass module shared by all 8 cores."""
    from contextlib import ExitStack

    from concourse import bacc, mybir
    import concourse.tile as tile

    nc = bacc.Bacc("TRN2", target_bir_lowering=False, debug=False)
    zr = nc.dram_tensor("zr", [N, D], mybir.dt.float32,
                        kind="ExternalInput").ap()
    out = nc.dram_tensor("out", [P, NI * NCB + 2 * NI], mybir.dt.float32,
                         kind="ExternalOutput").ap()
    colp = nc.dram_tensor("colp", [1, CSPAN[1] - CSPAN[0]],
                          mybir.dt.bfloat16, kind="ExternalOutput").ap()
    with tile.TileContext(nc) as tc:
        with ExitStack() as ctx:
            _kernel_body(ctx, tc, out, colp, zr)
    return nc


_NC = None


def _get_nc(finalized=True):
    global _NC
    if _NC is None:
        _NC = build_nc()
    if finalized and not _NC.is_finalized():
        _NC.finalize()
    return _NC


def make_in_maps(z_orig, z_augment):
    z = np.ascontiguousarray(
        np.concatenate([np.asarray(z_augment, dtype=np.float32),
                        np.asarray(z_orig, dtype=np.float32)], axis=0))
    return [{"zr": np.roll(z, -ROWS_PER_CORE * c, axis=0)}
            for c in range(N_CORES)]


def reduce_outputs(results):
    """Reassemble full row sums from row/column partials, finish the loss."""
    R = ROWS_PER_CORE
    rowsum = np.zeros(N, dtype=np.float64)
    pos = np.zeros(N, dtype=np.float64)
    selfexp = np.zeros(N, dtype=np.float64)
    colps = []
    for c in range(N_CORES):
        out = np.asarray(results[c]["out"], dtype=np.float64)  # [128, 40]
        colps.append(np.asarray(
            results[c]["colp"], dtype=np.float64).reshape(-1))  # [3072]
        # local row r = i*128 + p  <->  out[p, ...] column index i
        rp = out[:, 0:NI * NCB].reshape(P, NI, NCB).sum(axis=2)  # [p, i]
        rows = (c * R + np.arange(R)) % N
        rowsum[rows] = rp.T.reshape(R)
        pos[rows] = out[:, NI * NCB:NI * NCB + NI].T.reshape(R)
        selfexp[rows] = out[:, NI * NCB + NI:NI * NCB + 2 * NI].T.reshape(R)
    # colpart of core c' covers its local columns [1024:4096); its local
    # column j*1024 + r is global row ((c'+j)*1024 + r) % N
    for c in range(N_CORES):
        for j in (1, 2, 3):
            src = (c - j) % N_CORES
            rows = (c * R + np.arange(R)) % N
            rowsum[rows] += colps[src][(j - 1) * R:j * R]
    loss = (-2.0 * pos + np.log(rowsum - selfexp)).sum() / N
    return np.float32(loss)


def kernel(z_orig, z_augment):
    from concourse.bass_utils import run_bass_kernel_spmd

    nc = _get_nc()
    in_maps = make_in_maps(z_orig, z_augment)
    res = run_bass_kernel_spmd(nc, in_maps, core_ids=list(range(N_CORES)))
    return reduce_outputs(res.results)
